# revision 11
# baseline (speedup 1.0000x reference)
"""Differential attention Trainium2 kernel (8 NeuronCores, SPMD over heads).

Sharding: 16 heads x 2 batch -> 8 cores, 2 heads/core (both batches).
Each core projects q/k/v for its 2 heads over all tokens, runs both
32-dim softmaxes, combines a1 - lam*a2, applies AV and its slice of the
out-projection; the host sums the 8 partial outputs.

Device layout is feature-major throughout (no transposes on device):
  - logits are computed transposed [s, t] (s on partitions)
  - softmax denominators come from the same AV matmul via constant
    1.25-columns appended to v (every output partition of a matmul with a
    constant-column stationary operand receives the column sum -> a free
    partition-broadcast of 1.25*r; 1/1.25 = 0.8 folds the (1-lambda_init)
    output scale)
  - exp has a constant -4 shift for fp16 range safety (cancels in a/r)
  - lam is carried via a [128,1] input and folded with a negated out_proj
"""

import math
import os
import sys
import time

for _p in ("/opt/trn_rl_repo", "/root/.axon_site/_ro/trn_rl_repo"):
    if os.path.isdir(_p) and _p not in sys.path:
        sys.path.insert(0, _p)

import numpy as np

B, T, D = 2, 2048, 1024
H, HD, SD = 16, 64, 32
BT = B * T  # 4096
SCALING = 1.0 / math.sqrt(SD)
LAMBDA_INIT = 0.8 - 0.6 * math.exp(-0.3 * 0)  # 0.2
NCORES = 8
EXP_SHIFT = -4.0
CONST_COL = 1.25  # reciprocal of (1 - LAMBDA_INIT)

_NC_CACHE = {}
LAST_RESULTS = None


def _patch_tile_drain():
    """Split the TileContext tail-drain waits: this walrus build rejects >1
    sync wait on a TPB_CTRL instruction."""
    import bass_rust
    import concourse.tile as tile
    from concourse.tile import ScopedClock

    if getattr(tile.TileContext, "_drain_patched", False):
        return

    def _drain_and_barrier(self, tick_clock, wait_clock):
        nc = self.nc
        probe = nc.sync.nop()
        wait_clock.add_sem_waits(
            probe.ins, ScopedClock({None: tick_clock.global_clock})
        )
        si = probe.ins.sync_info
        waits = list(si.on_wait) if si is not None else []
        if len(waits) > 1:
            si.on_wait = waits[:1]
            for w in waits[1:]:
                n2 = nc.sync.nop()
                n2.ins.sync_info = bass_rust.SyncInfo(on_wait=[w], on_update=[])
        nc.sync.drain()
        nc.all_engine_barrier()
        assert self.sems is not None
        popped = nc._tile_sem_poison_stack.pop()
        assert popped is self._sem_poison
        nc.clear_and_free_semaphores(list(self.sems.allocated().values()))
        nc.all_engine_barrier()

    tile.TileContext._drain_and_barrier = _drain_and_barrier
    tile.TileContext._drain_patched = True


def _install_ntff_hook_shim():
    """Register the ctypes NTFF profiling hook that the boot script would
    have installed if antenv.axon_hooks existed in this checkout; also
    neuter the artifact upload (no bucket access needed for local dev)."""
    import contextlib
    import ctypes
    import types

    import concourse.bass_utils as bu

    bu.upload_artifacts = lambda tmpdir: "local://" + tmpdir

    try:
        from antenv import axon_hooks  # noqa: F401
        return
    except ImportError:
        pass

    so_path = "/opt/axon/libaxon_pjrt.so"
    if not os.path.exists(so_path):
        return
    lib = ctypes.CDLL(so_path)
    if not hasattr(lib, "axon_start_nrt_profile"):
        return
    lib.axon_start_nrt_profile.argtypes = [
        ctypes.POINTER(ctypes.c_int64), ctypes.c_size_t]
    lib.axon_start_nrt_profile.restype = ctypes.c_int64
    lib.axon_stop_nrt_profile.argtypes = [ctypes.c_char_p]
    lib.axon_stop_nrt_profile.restype = ctypes.c_int64

    @contextlib.contextmanager
    def _hook(output_dir, device_ids):
        import jax
        jax.devices()
        if device_ids:
            ids = (ctypes.c_int64 * len(device_ids))(*device_ids)
            rc = lib.axon_start_nrt_profile(ids, len(device_ids))
        else:
            rc = lib.axon_start_nrt_profile(None, 0)
        if rc != 0:
            raise RuntimeError(f"axon_start_nrt_profile rc={rc}")
        try:
            yield
        finally:
            n = lib.axon_stop_nrt_profile(str(output_dir).encode())
            print(f"ntff profile: {n} file(s) -> {output_dir}", file=sys.stderr)

    import antenv
    mod = types.ModuleType("antenv.axon_hooks")
    mod.get_axon_ntff_profile_hook = lambda: _hook
    sys.modules["antenv.axon_hooks"] = mod
    antenv.axon_hooks = mod


def _split_waits(nc, maxw=1):
    """This walrus build caps sync waits per instruction; hoist excess waits
    onto same-engine NoOps inserted right before the offending instruction
    (in-order engine queues make this semantics-preserving)."""
    import bass_rust
    import concourse.mybir as mybir

    nid = 0
    for f in nc.m.functions:
        for blk in f.blocks:
            out = []
            changed = False
            for inst in blk.instructions:
                si = inst.sync_info
                if si is not None:
                    waits = list(si.on_wait)
                    if len(waits) > maxw:
                        extra, keep = waits[:-maxw], waits[-maxw:]
                        for i in range(0, len(extra), maxw):
                            nop = mybir.InstNoOp(
                                name=f"wsplit-{nid}", ins=[], outs=[])
                            nid += 1
                            nop.engine = inst.engine
                            nop.sync_info = bass_rust.SyncInfo(
                                on_wait=extra[i:i + maxw], on_update=[])
                            out.append(nop)
                        si.on_wait = keep
                        changed = True
                out.append(inst)
            if changed:
                blk.instructions = out


def _build_nc():
    """Build the per-core Bass program (identical on all cores; data differs)."""
    import concourse.bass as bass
    import concourse.mybir as mybir
    import concourse.tile as tile
    from contextlib import ExitStack

    _patch_tile_drain()

    f16 = mybir.dt.float16
    f32 = mybir.dt.float32
    Exp = mybir.ActivationFunctionType.Exp
    mult = mybir.AluOpType.mult
    sub = mybir.AluOpType.subtract

    nc = bass.Bass("TRN2", target_bir_lowering=False, debug=False)

    xq_d = nc.dram_tensor("xq", [D, BT], f16, kind="ExternalInput").ap()
    xk_d = nc.dram_tensor("xk", [D, BT], f16, kind="ExternalInput").ap()
    xv_d = nc.dram_tensor("xv", [D, BT], f16, kind="ExternalInput").ap()
    wq_d = nc.dram_tensor("wq", [128, 1024], f16, kind="ExternalInput").ap()
    wk_d = nc.dram_tensor("wk", [128, 1024], f16, kind="ExternalInput").ap()
    wv_d = nc.dram_tensor("wv", [128, 1024], f16, kind="ExternalInput").ap()
    wo_d = nc.dram_tensor("wo", [128, 1024], f16, kind="ExternalInput").ap()
    lam_d = nc.dram_tensor("lamr", [128, 1], f32, kind="ExternalInput").ap()
    yT_d = nc.dram_tensor("yT", [D, BT], f16, kind="ExternalOutput").ap()

    mm = nc.tensor.matmul

    with tile.TileContext(nc) as tc, ExitStack() as ctx:
        wpool = ctx.enter_context(tc.tile_pool(name="w", bufs=1))
        xpool = ctx.enter_context(tc.tile_pool(name="x", bufs=16))
        qkpool = ctx.enter_context(tc.tile_pool(name="qk", bufs=1))
        vpool = ctx.enter_context(tc.tile_pool(name="v", bufs=1))
        epool = ctx.enter_context(tc.tile_pool(name="e", bufs=4))
        cpool = ctx.enter_context(tc.tile_pool(name="c", bufs=6))
        apool = ctx.enter_context(tc.tile_pool(name="a", bufs=16))
        ypool = ctx.enter_context(tc.tile_pool(name="y", bufs=4))

        wq_sb = wpool.tile([128, 1024], f16, tag="wq")
        wk_sb = wpool.tile([128, 1024], f16, tag="wk")
        wv_sb = wpool.tile([128, 1024], f16, tag="wv")
        wo_sb = wpool.tile([128, 1024], f16, tag="wo")
        lam_sb = wpool.tile([128, 1], f32, tag="lam")
        nc.sync.dma_start(wq_sb[:], wq_d[:])
        nc.sync.dma_start(wk_sb[:], wk_d[:])
        nc.sync.dma_start(wv_sb[:], wv_d[:])
        nc.sync.dma_start(wo_sb[:], wo_d[:])
        nc.sync.dma_start(lam_sb[:], lam_d[:])

        # qT/kT: [j=128 (2 heads x 64), t=4096] fp16; v: per 128-token chunk,
        # 4 slots of 64 cols: [v_h0 | 1.25 | v_h1 | 1.25]
        qT = qkpool.tile([128, BT], f16, tag="qT")
        kT = qkpool.tile([128, BT], f16, tag="kT")
        v_sb = vpool.tile([128, 32 * 256], f16, tag="v")
        v4 = v_sb[:].rearrange("p (s four f) -> p s four f", four=4, f=64)
        nc.vector.memset(v4[:, :, 1, :], CONST_COL)
        nc.vector.memset(v4[:, :, 3, :], CONST_COL)
        eshift = wpool.tile([128, 1], f32, tag="eshift")
        nc.vector.memset(eshift[:], EXP_SHIFT)

        at_tiles = []

        with (
            tc.tile_pool(name="psA", bufs=2, space="PSUM") as psA,
            tc.tile_pool(name="psB", bufs=1, space="PSUM") as psB,
        ):
            for b in range(B):
                th = b * 2048  # token offset of this batch half

                # ---- projections for this half ----
                xq_t = [xpool.tile([128, 2048], f16, tag="x", name=f"xq{b}_{k}") for k in range(8)]
                for k in range(8):
                    nc.sync.dma_start(
                        xq_t[k][:], xq_d[128 * k:128 * (k + 1), th:th + 2048]
                    )
                for ncb in range(4):
                    pq = psA.tile([128, 512], f32, tag="pl")
                    for k in range(8):
                        mm(pq[:], wq_sb[:, 128 * k:128 * (k + 1)],
                           xq_t[k][:, 512 * ncb:512 * (ncb + 1)],
                           start=(k == 0), stop=(k == 7))
                    nc.scalar.copy(qT[:, th + 512 * ncb:th + 512 * (ncb + 1)], pq[:])

                xk_t = [xpool.tile([128, 2048], f16, tag="x", name=f"xk{b}_{k}") for k in range(8)]
                for k in range(8):
                    nc.sync.dma_start(
                        xk_t[k][:], xk_d[128 * k:128 * (k + 1), th:th + 2048]
                    )
                for ncb in range(4):
                    pk = psA.tile([128, 512], f32, tag="pl")
                    for k in range(8):
                        mm(pk[:], wk_sb[:, 128 * k:128 * (k + 1)],
                           xk_t[k][:, 512 * ncb:512 * (ncb + 1)],
                           start=(k == 0), stop=(k == 7))
                    nc.vector.tensor_copy(kT[:, th + 512 * ncb:th + 512 * (ncb + 1)], pk[:])

                xv_t = [xpool.tile([128, 2048], f16, tag="x", name=f"xv{b}_{k}") for k in range(8)]
                for k in range(8):
                    nc.sync.dma_start(
                        xv_t[k][:], xv_d[128 * k:128 * (k + 1), th:th + 2048]
                    )
                for sc in range(16):
                    pv = psA.tile([128, 128], f32, tag="pl")
                    for k in range(8):
                        mm(pv[:], xv_t[k][:, 128 * sc:128 * (sc + 1)],
                           wv_sb[:, 128 * k:128 * (k + 1)],
                           start=(k == 0), stop=(k == 7))
                    scg = b * 16 + sc
                    nc.vector.tensor_copy(
                        v4[:, scg, 0, :], pv[:, 0:64])
                    nc.vector.tensor_copy(
                        v4[:, scg, 2, :], pv[:, 64:128])

                # ---- attention for this half ----
                for tcb in range(4):
                    qoff = th + 512 * tcb
                    e1 = psB.tile([128, 512], f32, tag="e1")
                    e2 = psB.tile([128, 512], f32, tag="e2")
                    r1 = psB.tile([128, 512], f32, tag="r1")
                    r2 = psB.tile([128, 512], f32, tag="r2")
                    for sc in range(16):
                        koff = th + 128 * sc
                        scg = b * 16 + sc
                        st = dict(start=(sc == 0), stop=(sc == 15))

                        pA = psA.tile([128, 1024], f32, tag="pl")
                        mm(pA[:, 0:512], kT[0:32, koff:koff + 128],
                           qT[0:32, qoff:qoff + 512], tile_position=(0, 0))
                        mm(pA[:, 512:1024], kT[32:64, koff:koff + 128],
                           qT[32:64, qoff:qoff + 512], tile_position=(32, 0))
                        eA = epool.tile([128, 1024], f16, tag="E")
                        nc.scalar.activation(eA[:], pA[:], Exp, bias=eshift[:])

                        pB = psA.tile([128, 1024], f32, tag="pl")
                        mm(pB[:, 0:512], kT[64:96, koff:koff + 128],
                           qT[64:96, qoff:qoff + 512], tile_position=(64, 0))
                        mm(pB[:, 512:1024], kT[96:128, koff:koff + 128],
                           qT[96:128, qoff:qoff + 512], tile_position=(96, 0))
                        eB = epool.tile([128, 1024], f16, tag="E")
                        nc.scalar.activation(eB[:], pB[:], Exp, bias=eshift[:])

                        vb = 256 * scg
                        v0 = v_sb[:, vb:vb + 64]
                        c0 = v_sb[:, vb + 64:vb + 128]
                        v1 = v_sb[:, vb + 128:vb + 192]
                        c1 = v_sb[:, vb + 192:vb + 256]
                        mm(e1[0:64, :], v0, eA[:, 0:512], **st)
                        mm(e1[64:128, :], v1, eB[:, 0:512], **st)
                        mm(e2[0:64, :], v0, eA[:, 512:1024], **st)
                        mm(e2[64:128, :], v1, eB[:, 512:1024], **st)
                        mm(r1[0:64, :], c0, eA[:, 0:512], **st)
                        mm(r1[64:128, :], c1, eB[:, 0:512], **st)
                        mm(r2[0:64, :], c0, eA[:, 512:1024], **st)
                        mm(r2[64:128, :], c1, eB[:, 512:1024], **st)

                    rec1 = cpool.tile([128, 512], f32, tag="cmb")
                    nc.vector.reciprocal(rec1[:], r1[:])
                    rec2 = cpool.tile([128, 512], f32, tag="cmb")
                    nc.vector.reciprocal(rec2[:], r2[:])
                    m1 = cpool.tile([128, 512], f32, tag="cmb")
                    nc.vector.tensor_tensor(m1[:], e1[:], rec1[:], mult)
                    m2 = cpool.tile([128, 512], f32, tag="cmb")
                    nc.vector.tensor_tensor(m2[:], e2[:], rec2[:], mult)
                    at = apool.tile([128, 512], f16, tag="at")
                    # at = lam*m2 - m1 = -(m1 - lam*m2); wo is pre-negated
                    nc.vector.scalar_tensor_tensor(
                        at[:], m2[:], lam_sb[:], m1[:], mult, sub)
                    at_tiles.append(at)

        # ---- out-projection (psum banks now free) ----
        with tc.tile_pool(name="psC", bufs=4, space="PSUM") as psC:
            for oc in range(8):
                woc = wo_sb[:, 128 * oc:128 * (oc + 1)]
                for pair in range(4):
                    b, tcp = divmod(pair, 2)
                    pt = psC.tile([128, 1024], f32, tag="pop")
                    mm(pt[:, 0:512], woc, at_tiles[b * 4 + tcp * 2][:])
                    mm(pt[:, 512:1024], woc, at_tiles[b * 4 + tcp * 2 + 1][:])
                    yt = ypool.tile([128, 1024], f16, tag="y")
                    if pair % 2 == 0:
                        nc.scalar.copy(yt[:], pt[:])
                    else:
                        nc.vector.tensor_copy(yt[:], pt[:])
                    toff = b * 2048 + tcp * 1024
                    nc.sync.dma_start(
                        yT_d[128 * oc:128 * (oc + 1), toff:toff + 1024], yt[:]
                    )

    _split_waits(nc)
    return nc


def _numpy_reference(query, key, value, in_proj_weight, in_proj_bias,
                     out_proj_weight, out_proj_bias,
                     lambda_q1, lambda_k1, lambda_q2, lambda_k2):
    """Exact fallback (used only if q-bias is nonzero)."""
    q = query.reshape(BT, D) @ in_proj_weight[:D].T + in_proj_bias[:D]
    k = key.reshape(BT, D) @ in_proj_weight[D:2 * D].T + in_proj_bias[D:2 * D]
    v = value.reshape(BT, D) @ in_proj_weight[2 * D:].T + in_proj_bias[2 * D:]
    q = q.reshape(B, T, H, HD).transpose(0, 2, 1, 3)
    k = k.reshape(B, T, H, HD).transpose(0, 2, 1, 3)
    v = v.reshape(B, T, H, HD).transpose(0, 2, 1, 3)
    lam = (math.exp(float(np.dot(lambda_q1, lambda_k1)))
           - math.exp(float(np.dot(lambda_q2, lambda_k2))) + LAMBDA_INIT)

    def smax(x):
        m = x.max(-1, keepdims=True)
        e = np.exp(x - m)
        return e / e.sum(-1, keepdims=True)

    a1 = smax(np.einsum("bhtd,bhsd->bhts", q[..., :SD], k[..., :SD]) * SCALING)
    a2 = smax(np.einsum("bhtd,bhsd->bhts", q[..., SD:], k[..., SD:]) * SCALING)
    o = np.einsum("bhts,bhsd->bhtd", a1 - lam * a2, v) * (1.0 - LAMBDA_INIT)
    o = o.transpose(0, 2, 1, 3).reshape(B, T, D)
    return (o @ out_proj_weight.T + out_proj_bias).astype(np.float32)


def kernel(**inputs):
    global LAST_RESULTS
    query = np.asarray(inputs["query"], dtype=np.float32)
    key = np.asarray(inputs["key"], dtype=np.float32)
    value = np.asarray(inputs["value"], dtype=np.float32)
    ipw = np.asarray(inputs["in_proj_weight"], dtype=np.float32)
    ipb = np.asarray(inputs["in_proj_bias"], dtype=np.float32)
    opw = np.asarray(inputs["out_proj_weight"], dtype=np.float32)
    opb = np.asarray(inputs["out_proj_bias"], dtype=np.float32)
    lq1 = np.asarray(inputs["lambda_q1"], dtype=np.float64)
    lk1 = np.asarray(inputs["lambda_k1"], dtype=np.float64)
    lq2 = np.asarray(inputs["lambda_q2"], dtype=np.float64)
    lk2 = np.asarray(inputs["lambda_k2"], dtype=np.float64)

    if np.any(ipb[:D] != 0.0):
        # q-bias changes the softmax itself; exact slow path
        return _numpy_reference(query, key, value, ipw, ipb, opw, opb,
                                lq1, lk1, lq2, lk2)

    lam = float(np.exp(np.dot(lq1, lk1)) - np.exp(np.dot(lq2, lk2))
                + LAMBDA_INIT)

    from concourse.bass_utils import run_bass_kernel_spmd

    if "nc" not in _NC_CACHE:
        _NC_CACHE["nc"] = _build_nc()
    nc = _NC_CACHE["nc"]

    xq = np.ascontiguousarray(query.reshape(BT, D).T).astype(np.float16)
    xk = np.ascontiguousarray(key.reshape(BT, D).T).astype(np.float16)
    xv = np.ascontiguousarray(value.reshape(BT, D).T).astype(np.float16)
    lam_arr = np.full((128, 1), lam, dtype=np.float32)

    qw, kw, vw = ipw[:D], ipw[D:2 * D], ipw[2 * D:]
    in_maps = []
    for c in range(NCORES):
        sl = slice(128 * c, 128 * (c + 1))

        def wlay(w):  # [128 rows, 1024 cols] -> [p, k*128+j] sbuf layout
            wT = np.ascontiguousarray(w.T)  # [1024, 128]
            return np.ascontiguousarray(
                wT.reshape(8, 128, 128).transpose(1, 0, 2).reshape(128, 1024)
            ).astype(np.float16)

        in_maps.append({
            "xq": xq, "xk": xk, "xv": xv,
            "wq": wlay(SCALING * qw[sl]),
            "wk": wlay(kw[sl]),
            "wv": wlay(vw[sl]),
            "wo": np.ascontiguousarray((-opw[:, sl]).T).astype(np.float16),
            "lamr": lam_arr,
        })

    do_trace = bool(os.environ.get("KERNEL_TRACE"))
    if do_trace:
        _install_ntff_hook_shim()
    res = run_bass_kernel_spmd(
        nc, in_maps, core_ids=list(range(NCORES)),
        trace=do_trace,
        tmpdir=os.environ.get("KERNEL_TRACE_DIR") or None,
    )
    LAST_RESULTS = res

    acc = np.zeros((D, BT), dtype=np.float32)
    for c in range(NCORES):
        acc += res.results[c]["yT"].astype(np.float32)
    y = np.ascontiguousarray(acc.T).reshape(B, T, D)

    # host-side exact bias corrections (biases are zero per spec, but cheap)
    vb = ipb[2 * D:]
    if np.any(vb != 0.0):
        y += (1.0 - LAMBDA_INIT) * (1.0 - lam) * (opw @ vb)
    if np.any(opb != 0.0):
        y += opb
    return y.astype(np.float32)


# revision 12
# speedup vs baseline: 1.1581x; 1.1581x over previous
"""Differential attention Trainium2 kernel (8 NeuronCores, SPMD over heads).

Sharding: 16 heads x 2 batch -> 8 cores, 2 heads/core (both batches).
Each core projects q/k/v for its 2 heads over all tokens, runs both
32-dim softmaxes, combines a1 - lam*a2, applies AV and its slice of the
out-projection; the host sums the 8 partial outputs.

Device layout is feature-major throughout (no transposes on device):
  - logits are computed transposed [s, t] (s on partitions)
  - softmax denominators come from the same AV matmul via constant
    1.25-columns appended to v (every output partition of a matmul with a
    constant-column stationary operand receives the column sum -> a free
    partition-broadcast of 1.25*r; 1/1.25 = 0.8 folds the (1-lambda_init)
    output scale)
  - exp has a constant -4 shift for fp16 range safety (cancels in a/r)
  - lam is carried via a [128,1] input and folded with a negated out_proj
"""

import math
import os
import sys
import time

for _p in ("/opt/trn_rl_repo", "/root/.axon_site/_ro/trn_rl_repo"):
    if os.path.isdir(_p) and _p not in sys.path:
        sys.path.insert(0, _p)

import numpy as np

B, T, D = 2, 2048, 1024
H, HD, SD = 16, 64, 32
BT = B * T  # 4096
SCALING = 1.0 / math.sqrt(SD)
LAMBDA_INIT = 0.8 - 0.6 * math.exp(-0.3 * 0)  # 0.2
NCORES = 8
EXP_SHIFT = -4.0
CONST_COL = 1.25  # reciprocal of (1 - LAMBDA_INIT)

_NC_CACHE = {}
LAST_RESULTS = None


def _patch_tile_drain():
    """Split the TileContext tail-drain waits: this walrus build rejects >1
    sync wait on a TPB_CTRL instruction."""
    import bass_rust
    import concourse.tile as tile
    from concourse.tile import ScopedClock

    if getattr(tile.TileContext, "_drain_patched", False):
        return

    def _drain_and_barrier(self, tick_clock, wait_clock):
        nc = self.nc
        probe = nc.sync.nop()
        wait_clock.add_sem_waits(
            probe.ins, ScopedClock({None: tick_clock.global_clock})
        )
        si = probe.ins.sync_info
        waits = list(si.on_wait) if si is not None else []
        if len(waits) > 1:
            si.on_wait = waits[:1]
            for w in waits[1:]:
                n2 = nc.sync.nop()
                n2.ins.sync_info = bass_rust.SyncInfo(on_wait=[w], on_update=[])
        nc.sync.drain()
        nc.all_engine_barrier()
        assert self.sems is not None
        popped = nc._tile_sem_poison_stack.pop()
        assert popped is self._sem_poison
        nc.clear_and_free_semaphores(list(self.sems.allocated().values()))
        nc.all_engine_barrier()

    tile.TileContext._drain_and_barrier = _drain_and_barrier
    tile.TileContext._drain_patched = True


def _install_ntff_hook_shim():
    """Register the ctypes NTFF profiling hook that the boot script would
    have installed if antenv.axon_hooks existed in this checkout; also
    neuter the artifact upload (no bucket access needed for local dev)."""
    import contextlib
    import ctypes
    import types

    import concourse.bass_utils as bu

    bu.upload_artifacts = lambda tmpdir: "local://" + tmpdir

    try:
        from antenv import axon_hooks  # noqa: F401
        return
    except ImportError:
        pass

    so_path = "/opt/axon/libaxon_pjrt.so"
    if not os.path.exists(so_path):
        return
    lib = ctypes.CDLL(so_path)
    if not hasattr(lib, "axon_start_nrt_profile"):
        return
    lib.axon_start_nrt_profile.argtypes = [
        ctypes.POINTER(ctypes.c_int64), ctypes.c_size_t]
    lib.axon_start_nrt_profile.restype = ctypes.c_int64
    lib.axon_stop_nrt_profile.argtypes = [ctypes.c_char_p]
    lib.axon_stop_nrt_profile.restype = ctypes.c_int64

    @contextlib.contextmanager
    def _hook(output_dir, device_ids):
        import jax
        jax.devices()
        if device_ids:
            ids = (ctypes.c_int64 * len(device_ids))(*device_ids)
            rc = lib.axon_start_nrt_profile(ids, len(device_ids))
        else:
            rc = lib.axon_start_nrt_profile(None, 0)
        if rc != 0:
            raise RuntimeError(f"axon_start_nrt_profile rc={rc}")
        try:
            yield
        finally:
            n = lib.axon_stop_nrt_profile(str(output_dir).encode())
            print(f"ntff profile: {n} file(s) -> {output_dir}", file=sys.stderr)

    import antenv
    mod = types.ModuleType("antenv.axon_hooks")
    mod.get_axon_ntff_profile_hook = lambda: _hook
    sys.modules["antenv.axon_hooks"] = mod
    antenv.axon_hooks = mod


def _split_waits(nc, maxw=1):
    """This walrus build caps sync waits per instruction; hoist excess waits
    onto same-engine NoOps inserted right before the offending instruction
    (in-order engine queues make this semantics-preserving)."""
    import bass_rust
    import concourse.mybir as mybir

    nid = 0
    for f in nc.m.functions:
        for blk in f.blocks:
            out = []
            changed = False
            for inst in blk.instructions:
                si = inst.sync_info
                if si is not None:
                    waits = list(si.on_wait)
                    if len(waits) > maxw:
                        extra, keep = waits[:-maxw], waits[-maxw:]
                        for i in range(0, len(extra), maxw):
                            nop = mybir.InstNoOp(
                                name=f"wsplit-{nid}", ins=[], outs=[])
                            nid += 1
                            nop.engine = inst.engine
                            nop.sync_info = bass_rust.SyncInfo(
                                on_wait=extra[i:i + maxw], on_update=[])
                            out.append(nop)
                        si.on_wait = keep
                        changed = True
                out.append(inst)
            if changed:
                blk.instructions = out


def _build_nc():
    """Build the per-core Bass program (identical on all cores; data differs)."""
    import concourse.bass as bass
    import concourse.mybir as mybir
    import concourse.tile as tile
    from contextlib import ExitStack

    _patch_tile_drain()

    f16 = mybir.dt.float16
    f32 = mybir.dt.float32
    Exp = mybir.ActivationFunctionType.Exp
    mult = mybir.AluOpType.mult
    sub = mybir.AluOpType.subtract

    nc = bass.Bass("TRN2", target_bir_lowering=False, debug=False)

    xq_d = nc.dram_tensor("xq", [D, BT], f16, kind="ExternalInput").ap()
    xk_d = nc.dram_tensor("xk", [D, BT], f16, kind="ExternalInput").ap()
    xv_d = nc.dram_tensor("xv", [D, BT], f16, kind="ExternalInput").ap()
    wq_d = nc.dram_tensor("wq", [128, 1024], f16, kind="ExternalInput").ap()
    wk_d = nc.dram_tensor("wk", [128, 1024], f16, kind="ExternalInput").ap()
    wv_d = nc.dram_tensor("wv", [128, 1024], f16, kind="ExternalInput").ap()
    wo_d = nc.dram_tensor("wo", [128, 1024], f16, kind="ExternalInput").ap()
    lam_d = nc.dram_tensor("lamr", [128, 1], f32, kind="ExternalInput").ap()
    yT_d = nc.dram_tensor("yT", [D, BT], f16, kind="ExternalOutput").ap()

    mm = nc.tensor.matmul

    with tile.TileContext(nc) as tc, ExitStack() as ctx:
        wpool = ctx.enter_context(tc.tile_pool(name="w", bufs=1))
        xpool = ctx.enter_context(tc.tile_pool(name="x", bufs=26))
        qkpool = ctx.enter_context(tc.tile_pool(name="qk", bufs=1))
        vpool = ctx.enter_context(tc.tile_pool(name="v", bufs=1))
        epool = ctx.enter_context(tc.tile_pool(name="e", bufs=4))
        cpool = ctx.enter_context(tc.tile_pool(name="c", bufs=14))
        apool = ctx.enter_context(tc.tile_pool(name="a", bufs=16))
        ypool = ctx.enter_context(tc.tile_pool(name="y", bufs=4))

        wq_sb = wpool.tile([128, 1024], f16, tag="wq")
        wk_sb = wpool.tile([128, 1024], f16, tag="wk")
        wv_sb = wpool.tile([128, 1024], f16, tag="wv")
        wo_sb = wpool.tile([128, 1024], f16, tag="wo")
        lam_sb = wpool.tile([128, 1], f32, tag="lam")
        nc.sync.dma_start(wq_sb[:], wq_d[:])
        nc.sync.dma_start(wk_sb[:], wk_d[:])
        nc.sync.dma_start(wv_sb[:], wv_d[:])
        nc.sync.dma_start(wo_sb[:], wo_d[:])
        nc.sync.dma_start(lam_sb[:], lam_d[:])

        # qT/kT: [j=128 (2 heads x 64), t=4096] fp16; v: per 128-token chunk,
        # 4 slots of 64 cols: [v_h0 | 1.25 | v_h1 | 1.25]
        qT = qkpool.tile([128, BT], f16, tag="qT")
        kT = qkpool.tile([128, BT], f16, tag="kT")
        v_sb = vpool.tile([128, 32 * 192], f16, tag="v")
        v4 = v_sb[:].rearrange("p (s three f) -> p s three f", three=3, f=64)
        nc.vector.memset(v4[:, :, 1, :], CONST_COL)
        eshift = wpool.tile([128, 1], f32, tag="eshift")
        nc.vector.memset(eshift[:], EXP_SHIFT)

        at_tiles = []

        with (
            tc.tile_pool(name="psA", bufs=2, space="PSUM") as psA,
            tc.tile_pool(name="psB", bufs=1, space="PSUM") as psB,
        ):
            for b in range(B):
                th = b * 2048  # token offset of this batch half

                # ---- projections for this half ----
                xq_t = [xpool.tile([128, 2048], f16, tag="x", name=f"xq{b}_{k}") for k in range(8)]
                for k in range(8):
                    nc.sync.dma_start(
                        xq_t[k][:], xq_d[128 * k:128 * (k + 1), th:th + 2048]
                    )
                for ncb in range(4):
                    pq = psA.tile([128, 512], f32, tag="pl")
                    for k in range(8):
                        mm(pq[:], wq_sb[:, 128 * k:128 * (k + 1)],
                           xq_t[k][:, 512 * ncb:512 * (ncb + 1)],
                           start=(k == 0), stop=(k == 7))
                    nc.scalar.copy(qT[:, th + 512 * ncb:th + 512 * (ncb + 1)], pq[:])

                xk_t = [xpool.tile([128, 2048], f16, tag="x", name=f"xk{b}_{k}") for k in range(8)]
                for k in range(8):
                    nc.sync.dma_start(
                        xk_t[k][:], xk_d[128 * k:128 * (k + 1), th:th + 2048]
                    )
                for ncb in range(4):
                    pk = psA.tile([128, 512], f32, tag="pl")
                    for k in range(8):
                        mm(pk[:], wk_sb[:, 128 * k:128 * (k + 1)],
                           xk_t[k][:, 512 * ncb:512 * (ncb + 1)],
                           start=(k == 0), stop=(k == 7))
                    nc.vector.tensor_copy(kT[:, th + 512 * ncb:th + 512 * (ncb + 1)], pk[:])

                xv_t = [xpool.tile([128, 2048], f16, tag="x", name=f"xv{b}_{k}") for k in range(8)]
                for k in range(8):
                    nc.sync.dma_start(
                        xv_t[k][:], xv_d[128 * k:128 * (k + 1), th:th + 2048]
                    )
                for sc in range(16):
                    pv = psA.tile([128, 128], f32, tag="pl")
                    for k in range(8):
                        mm(pv[:], xv_t[k][:, 128 * sc:128 * (sc + 1)],
                           wv_sb[:, 128 * k:128 * (k + 1)],
                           start=(k == 0), stop=(k == 7))
                    scg = b * 16 + sc
                    nc.vector.tensor_copy(
                        v4[:, scg, 0, :], pv[:, 0:64])
                    nc.vector.tensor_copy(
                        v4[:, scg, 2, :], pv[:, 64:128])

                # ---- attention for this half ----
                for tcb in range(4):
                    qoff = th + 512 * tcb
                    # Q1=[av1h0|r1h0]  Q2=[r1h1|av1h1]  Q3=[av2h0|r2h0]  Q4=[r2h1|av2h1]
                    q1b = psB.tile([128, 512], f32, tag="e1")
                    q2b = psB.tile([128, 512], f32, tag="e2")
                    q3b = psB.tile([128, 512], f32, tag="r1")
                    q4b = psB.tile([128, 512], f32, tag="r2")
                    for sc in range(16):
                        koff = th + 128 * sc
                        scg = b * 16 + sc
                        st = dict(start=(sc == 0), stop=(sc == 15))

                        pA = psA.tile([128, 1024], f32, tag="pl")
                        mm(pA[:, 0:512], kT[0:32, koff:koff + 128],
                           qT[0:32, qoff:qoff + 512], tile_position=(0, 0))
                        mm(pA[:, 512:1024], kT[32:64, koff:koff + 128],
                           qT[32:64, qoff:qoff + 512], tile_position=(32, 0))
                        eA = epool.tile([128, 1024], f16, tag="E")
                        nc.scalar.activation(eA[:], pA[:], Exp, bias=eshift[:])

                        pB = psA.tile([128, 1024], f32, tag="pl")
                        mm(pB[:, 0:512], kT[64:96, koff:koff + 128],
                           qT[64:96, qoff:qoff + 512], tile_position=(64, 0))
                        mm(pB[:, 512:1024], kT[96:128, koff:koff + 128],
                           qT[96:128, qoff:qoff + 512], tile_position=(96, 0))
                        eB = epool.tile([128, 1024], f16, tag="E")
                        nc.scalar.activation(eB[:], pB[:], Exp, bias=eshift[:])

                        vb = 192 * scg
                        lh0 = v_sb[:, vb:vb + 128]        # [v_h0 | 1.25]
                        lh1 = v_sb[:, vb + 64:vb + 192]   # [1.25 | v_h1]
                        mm(q1b[:], lh0, eA[:, 0:512], **st)
                        mm(q3b[:], lh0, eA[:, 512:1024], **st)
                        mm(q2b[:], lh1, eB[:, 0:512], **st)
                        mm(q4b[:], lh1, eB[:, 512:1024], **st)

                    # evacuate av/r halves base-preserving, then DMA
                    # swaps the r halves across partitions for alignment
                    avc1 = cpool.tile([128, 512], f32, tag="cmb")
                    nc.vector.tensor_copy(avc1[0:64, :], q1b[0:64, :])
                    nc.vector.tensor_copy(avc1[64:128, :], q2b[64:128, :])
                    avc2 = cpool.tile([128, 512], f32, tag="cmb")
                    nc.vector.tensor_copy(avc2[0:64, :], q3b[0:64, :])
                    nc.vector.tensor_copy(avc2[64:128, :], q4b[64:128, :])
                    rs1 = cpool.tile([128, 512], f32, tag="cmb")
                    nc.vector.tensor_copy(rs1[64:128, :], q1b[64:128, :])
                    nc.vector.tensor_copy(rs1[0:64, :], q2b[0:64, :])
                    rs2 = cpool.tile([128, 512], f32, tag="cmb")
                    nc.vector.tensor_copy(rs2[64:128, :], q3b[64:128, :])
                    nc.vector.tensor_copy(rs2[0:64, :], q4b[0:64, :])
                    rc1 = cpool.tile([128, 512], f32, tag="cmb")
                    nc.sync.dma_start(rc1[0:64, :], rs1[64:128, :])
                    nc.sync.dma_start(rc1[64:128, :], rs1[0:64, :])
                    rc2 = cpool.tile([128, 512], f32, tag="cmb")
                    nc.sync.dma_start(rc2[0:64, :], rs2[64:128, :])
                    nc.sync.dma_start(rc2[64:128, :], rs2[0:64, :])
                    rec1 = cpool.tile([128, 512], f32, tag="cmb")
                    nc.vector.reciprocal(rec1[:], rc1[:])
                    rec2 = cpool.tile([128, 512], f32, tag="cmb")
                    nc.vector.reciprocal(rec2[:], rc2[:])
                    m1 = cpool.tile([128, 512], f32, tag="cmb")
                    nc.vector.tensor_tensor(m1[:], avc1[:], rec1[:], mult)
                    m2 = cpool.tile([128, 512], f32, tag="cmb")
                    nc.vector.tensor_tensor(m2[:], avc2[:], rec2[:], mult)
                    at = apool.tile([128, 512], f16, tag="at")
                    # at = lam*m2 - m1 = -(m1 - lam*m2); wo is pre-negated
                    nc.vector.scalar_tensor_tensor(
                        at[:], m2[:], lam_sb[:], m1[:], mult, sub)
                    at_tiles.append(at)

                # ---- out-projection for this half (overlaps next phase) ----
                for oc in range(8):
                    woc = wo_sb[:, 128 * oc:128 * (oc + 1)]
                    for tcp in range(2):
                        pt = psA.tile([128, 1024], f32, tag="pl")
                        mm(pt[:, 0:512], woc, at_tiles[b * 4 + tcp * 2][:])
                        mm(pt[:, 512:1024], woc, at_tiles[b * 4 + tcp * 2 + 1][:])
                        yt = ypool.tile([128, 1024], f16, tag="y")
                        if (oc + tcp) % 2 == 0:
                            nc.scalar.copy(yt[:], pt[:])
                        else:
                            nc.vector.tensor_copy(yt[:], pt[:])
                        toff = th + tcp * 1024
                        nc.sync.dma_start(
                            yT_d[128 * oc:128 * (oc + 1), toff:toff + 1024],
                            yt[:])

    _split_waits(nc)
    return nc


def _numpy_reference(query, key, value, in_proj_weight, in_proj_bias,
                     out_proj_weight, out_proj_bias,
                     lambda_q1, lambda_k1, lambda_q2, lambda_k2):
    """Exact fallback (used only if q-bias is nonzero)."""
    q = query.reshape(BT, D) @ in_proj_weight[:D].T + in_proj_bias[:D]
    k = key.reshape(BT, D) @ in_proj_weight[D:2 * D].T + in_proj_bias[D:2 * D]
    v = value.reshape(BT, D) @ in_proj_weight[2 * D:].T + in_proj_bias[2 * D:]
    q = q.reshape(B, T, H, HD).transpose(0, 2, 1, 3)
    k = k.reshape(B, T, H, HD).transpose(0, 2, 1, 3)
    v = v.reshape(B, T, H, HD).transpose(0, 2, 1, 3)
    lam = (math.exp(float(np.dot(lambda_q1, lambda_k1)))
           - math.exp(float(np.dot(lambda_q2, lambda_k2))) + LAMBDA_INIT)

    def smax(x):
        m = x.max(-1, keepdims=True)
        e = np.exp(x - m)
        return e / e.sum(-1, keepdims=True)

    a1 = smax(np.einsum("bhtd,bhsd->bhts", q[..., :SD], k[..., :SD]) * SCALING)
    a2 = smax(np.einsum("bhtd,bhsd->bhts", q[..., SD:], k[..., SD:]) * SCALING)
    o = np.einsum("bhts,bhsd->bhtd", a1 - lam * a2, v) * (1.0 - LAMBDA_INIT)
    o = o.transpose(0, 2, 1, 3).reshape(B, T, D)
    return (o @ out_proj_weight.T + out_proj_bias).astype(np.float32)


def kernel(**inputs):
    global LAST_RESULTS
    query = np.asarray(inputs["query"], dtype=np.float32)
    key = np.asarray(inputs["key"], dtype=np.float32)
    value = np.asarray(inputs["value"], dtype=np.float32)
    ipw = np.asarray(inputs["in_proj_weight"], dtype=np.float32)
    ipb = np.asarray(inputs["in_proj_bias"], dtype=np.float32)
    opw = np.asarray(inputs["out_proj_weight"], dtype=np.float32)
    opb = np.asarray(inputs["out_proj_bias"], dtype=np.float32)
    lq1 = np.asarray(inputs["lambda_q1"], dtype=np.float64)
    lk1 = np.asarray(inputs["lambda_k1"], dtype=np.float64)
    lq2 = np.asarray(inputs["lambda_q2"], dtype=np.float64)
    lk2 = np.asarray(inputs["lambda_k2"], dtype=np.float64)

    if np.any(ipb[:D] != 0.0):
        # q-bias changes the softmax itself; exact slow path
        return _numpy_reference(query, key, value, ipw, ipb, opw, opb,
                                lq1, lk1, lq2, lk2)

    lam = float(np.exp(np.dot(lq1, lk1)) - np.exp(np.dot(lq2, lk2))
                + LAMBDA_INIT)

    from concourse.bass_utils import run_bass_kernel_spmd

    if "nc" not in _NC_CACHE:
        _NC_CACHE["nc"] = _build_nc()
    nc = _NC_CACHE["nc"]

    xq = np.ascontiguousarray(query.reshape(BT, D).T).astype(np.float16)
    xk = np.ascontiguousarray(key.reshape(BT, D).T).astype(np.float16)
    xv = np.ascontiguousarray(value.reshape(BT, D).T).astype(np.float16)
    lam_arr = np.full((128, 1), lam, dtype=np.float32)

    qw, kw, vw = ipw[:D], ipw[D:2 * D], ipw[2 * D:]
    in_maps = []
    for c in range(NCORES):
        sl = slice(128 * c, 128 * (c + 1))

        def wlay(w):  # [128 rows, 1024 cols] -> [p, k*128+j] sbuf layout
            wT = np.ascontiguousarray(w.T)  # [1024, 128]
            return np.ascontiguousarray(
                wT.reshape(8, 128, 128).transpose(1, 0, 2).reshape(128, 1024)
            ).astype(np.float16)

        in_maps.append({
            "xq": xq, "xk": xk, "xv": xv,
            "wq": wlay(SCALING * qw[sl]),
            "wk": wlay(kw[sl]),
            "wv": wlay(vw[sl]),
            "wo": np.ascontiguousarray((-opw[:, sl]).T).astype(np.float16),
            "lamr": lam_arr,
        })

    do_trace = bool(os.environ.get("KERNEL_TRACE"))
    if do_trace:
        _install_ntff_hook_shim()
    res = run_bass_kernel_spmd(
        nc, in_maps, core_ids=list(range(NCORES)),
        trace=do_trace,
        tmpdir=os.environ.get("KERNEL_TRACE_DIR") or None,
    )
    LAST_RESULTS = res

    acc = np.zeros((D, BT), dtype=np.float32)
    for c in range(NCORES):
        acc += res.results[c]["yT"].astype(np.float32)
    y = np.ascontiguousarray(acc.T).reshape(B, T, D)

    # host-side exact bias corrections (biases are zero per spec, but cheap)
    vb = ipb[2 * D:]
    if np.any(vb != 0.0):
        y += (1.0 - LAMBDA_INIT) * (1.0 - lam) * (opw @ vb)
    if np.any(opb != 0.0):
        y += opb
    return y.astype(np.float32)
